# revision 1
# baseline (speedup 1.0000x reference)
"""Trainium2 Bass kernel for per-expert MLP (MoE experts, expert-parallel).

Computes out = relu(relu(x @ w1) @ w2) per expert.
  x:  [E=32, N=1024, D_IN=3072] f32
  w1: [E, D_IN, D_H=1024] f32
  w2: [E, D_H, D_OUT=256] f32
  out:[E, N, D_OUT] f32

Sharding: expert dim E=32 split across 8 cores (4 experts/core), no
communication. Host pre-casts to bf16 and pre-tiles layouts so every DMA is
a plain partition-major copy and no on-chip transposes are needed:

GEMM1 computes hiddenT (h on partitions) directly:
  hiddenT[h, n] = sum_d w1[d, h] * x[n, d]
  lhsT = w1 tile [d(128 part), h(128 cols)]   (stationary)
  rhs  = xT tile [d(128 part), n(512 free)]   (moving)
GEMM2 then has contraction dim h already on partitions:
  out[n, o] = sum_h hiddenT[h, n] * w2[h, o]
  lhsT = hiddenT tile [h(128), n(128)], rhs = w2 tile [h(128), o(256)]
"""

import numpy as np
import ml_dtypes

E, N, D_IN, D_H, D_OUT = 32, 1024, 3072, 1024, 256
NCORES = 8
E_PER = E // NCORES  # 4 experts per core
P = 128
DT = D_IN // P  # 24 k-tiles for GEMM1
HT = D_H // P   # 8 h-tiles
NT = N // P     # 8 n-tiles
FD = 512        # matmul free dim (one PSUM bank of f32)
NCH = N // FD   # 2 n-chunks in GEMM1

_BF16 = ml_dtypes.bfloat16
_CACHE = {}


def _build_program():
    """Build + compile the per-core Bass program (same program on all cores)."""
    if "nc" in _CACHE:
        return _CACHE["nc"], _CACHE["names"]

    from contextlib import ExitStack

    import concourse.bass as bass
    import concourse.tile as tile
    from concourse import bacc, mybir

    bf16 = mybir.dt.bfloat16
    f32 = mybir.dt.float32

    nc = bacc.Bacc("TRN2", target_bir_lowering=False, debug=False,
                   enable_asserts=False)

    # Per-core DRAM I/O (host-prepped layouts, see kernel() below).
    x_d = nc.dram_tensor("xt", [E_PER, P, DT, N], bf16, kind="ExternalInput").ap()
    w1_d = nc.dram_tensor("w1t", [E_PER, HT, P, DT * P], bf16,
                          kind="ExternalInput").ap()
    w2_d = nc.dram_tensor("w2t", [E_PER, P, HT, D_OUT], bf16,
                          kind="ExternalInput").ap()
    out_d = nc.dram_tensor("out", [E_PER, N, D_OUT], f32,
                           kind="ExternalOutput").ap()

    relu = mybir.ActivationFunctionType.Relu

    with tile.TileContext(nc) as tc, ExitStack() as ctx:
        xp = ctx.enter_context(tc.tile_pool(name="x", bufs=2))
        w1p = ctx.enter_context(tc.tile_pool(name="w1", bufs=3))
        w2p = ctx.enter_context(tc.tile_pool(name="w2", bufs=2))
        hp = ctx.enter_context(tc.tile_pool(name="hid", bufs=2))
        op = ctx.enter_context(tc.tile_pool(name="o", bufs=4))
        ps1 = ctx.enter_context(tc.tile_pool(name="ps1", bufs=4, space="PSUM"))
        ps2 = ctx.enter_context(tc.tile_pool(name="ps2", bufs=2, space="PSUM"))

        for e in range(E_PER):
            # xT for this expert: [128, 24, 1024], one DMA per d-tile so
            # GEMM1 can start before the whole expert is resident.
            x_sb = xp.tile([P, DT, N], bf16, tag="x")
            for d in range(DT):
                nc.sync.dma_start(x_sb[:, d, :], x_d[e, :, d, :])
            w2_sb = w2p.tile([P, HT, D_OUT], bf16, tag="w2")
            nc.sync.dma_start(w2_sb[:], w2_d[e])

            hid = hp.tile([P, HT, N], bf16, tag="hid")

            # GEMM1 + relu -> hiddenT (bf16)
            for h in range(HT):
                w1_sb = w1p.tile([P, DT * P], bf16, tag="w1")
                nc.sync.dma_start(w1_sb[:], w1_d[e, h])
                pa = ps1.tile([P, FD], f32, tag="ps1")
                pb = ps1.tile([P, FD], f32, tag="ps1")
                for d in range(DT):
                    lhsT = w1_sb[:, bass.ts(d, P)]
                    nc.tensor.matmul(pa[:], lhsT, x_sb[:, d, 0:FD],
                                     start=(d == 0), stop=(d == DT - 1))
                    nc.tensor.matmul(pb[:], lhsT, x_sb[:, d, FD:N],
                                     start=(d == 0), stop=(d == DT - 1))
                nc.scalar.activation(hid[:, h, 0:FD], pa[:], relu)
                nc.scalar.activation(hid[:, h, FD:N], pb[:], relu)

            # GEMM2 + relu -> out rows
            for nt in range(NT):
                po = ps2.tile([P, D_OUT], f32, tag="ps2")
                for k in range(HT):
                    nc.tensor.matmul(po[:], hid[:, k, bass.ts(nt, P)],
                                     w2_sb[:, k, :],
                                     start=(k == 0), stop=(k == HT - 1))
                o_sb = op.tile([P, D_OUT], f32, tag="o")
                nc.scalar.activation(o_sb[:], po[:], relu)
                nc.sync.dma_start(out_d[e, bass.ds(nt * P, P), :], o_sb[:])

    nc.compile()
    _CACHE["nc"] = nc
    _CACHE["names"] = ("xt", "w1t", "w2t", "out")
    return nc, _CACHE["names"]


def _prep_inputs(x: np.ndarray, w1: np.ndarray, w2: np.ndarray):
    """Shard across cores + cast bf16 + pre-tile so all DMAs are contiguous."""
    # xT, partition-major: xt[e, p, d, n] = x[e, n, d*128+p]
    xt = np.ascontiguousarray(
        x.astype(_BF16).transpose(0, 2, 1)      # [E, D_IN, N]
        .reshape(E, DT, P, N).transpose(0, 2, 1, 3))  # [E, P, DT, N]
    # w1 h-tiled, partition-major: w1t[e, h, p, dt*128+c] = w1[e, dt*128+p, h*128+c]
    w1t = np.ascontiguousarray(
        w1.astype(_BF16).reshape(E, DT, P, HT, P)
        .transpose(0, 3, 2, 1, 4).reshape(E, HT, P, DT * P))
    # w2 k-tiled, partition-major: w2t[e, p, k, o] = w2[e, k*128+p, o]
    w2t = np.ascontiguousarray(
        w2.astype(_BF16).reshape(E, HT, P, D_OUT).transpose(0, 2, 1, 3))

    in_maps = []
    for c in range(NCORES):
        sl = slice(c * E_PER, (c + 1) * E_PER)
        in_maps.append({"xt": xt[sl], "w1t": w1t[sl], "w2t": w2t[sl]})
    return in_maps


def run(x, w1, w2, trace=False, **trace_kwargs):
    """Run on 8 cores; returns (full_out, BassKernelResults)."""
    from concourse.bass_utils import run_bass_kernel_spmd

    nc, _ = _build_program()
    in_maps = _prep_inputs(np.asarray(x), np.asarray(w1), np.asarray(w2))
    res = run_bass_kernel_spmd(nc, in_maps, list(range(NCORES)), trace=trace,
                               **trace_kwargs)
    out = np.concatenate([res.results[c]["out"] for c in range(NCORES)], axis=0)
    return out, res


def kernel(x: np.ndarray, w1: np.ndarray, w2: np.ndarray) -> np.ndarray:
    out, _ = run(x, w1, w2, trace=False)
    return out


# revision 3
# speedup vs baseline: 1.0327x; 1.0327x over previous
"""Trainium2 Bass kernel for per-expert MLP (MoE experts, expert-parallel).

Computes out = relu(relu(x @ w1) @ w2) per expert.
  x:  [E=32, N=1024, D_IN=3072] f32
  w1: [E, D_IN, D_H=1024] f32
  w2: [E, D_H, D_OUT=256] f32
  out:[E, N, D_OUT] f32

Sharding: expert dim E=32 split across 8 cores (4 experts/core), no
communication. Host pre-casts to bf16 and pre-tiles layouts so every DMA is
a plain partition-major copy and no on-chip transposes are needed:

GEMM1 computes hiddenT (h on partitions) directly:
  hiddenT[h, n] = sum_d w1[d, h] * x[n, d]
  lhsT = w1 tile [d(128 part), h(128 cols)]   (stationary)
  rhs  = xT tile [d(128 part), n(512 free)]   (moving)
GEMM2 then has contraction dim h already on partitions:
  out[n, o] = sum_h hiddenT[h, n] * w2[h, o]
  lhsT = hiddenT tile [h(128), n(128)], rhs = w2 tile [h(128), o(256)]
"""

import numpy as np
import ml_dtypes

E, N, D_IN, D_H, D_OUT = 32, 1024, 3072, 1024, 256
NCORES = 8
E_PER = E // NCORES  # 4 experts per core
P = 128
DT = D_IN // P  # 24 k-tiles for GEMM1
HT = D_H // P   # 8 h-tiles
NT = N // P     # 8 n-tiles
FD = 512        # matmul free dim (one PSUM bank of f32)
NCH = N // FD   # 2 n-chunks in GEMM1

_BF16 = ml_dtypes.bfloat16
_CACHE = {}


def _build_program():
    """Build + compile the per-core Bass program (same program on all cores)."""
    if "nc" in _CACHE:
        return _CACHE["nc"], _CACHE["names"]

    from contextlib import ExitStack

    import concourse.bass as bass
    import concourse.tile as tile
    from concourse import bacc, mybir

    bf16 = mybir.dt.bfloat16
    f32 = mybir.dt.float32

    nc = bacc.Bacc("TRN2", target_bir_lowering=False, debug=False,
                   enable_asserts=False)

    # Per-core DRAM I/O (host-prepped layouts, see kernel() below).
    x_d = nc.dram_tensor("xt", [E_PER, P, DT, N], bf16, kind="ExternalInput").ap()
    w1_d = nc.dram_tensor("w1t", [E_PER, HT, P, DT * P], bf16,
                          kind="ExternalInput").ap()
    w2_d = nc.dram_tensor("w2t", [E_PER, P, HT, D_OUT], bf16,
                          kind="ExternalInput").ap()
    out_d = nc.dram_tensor("out", [E_PER, N, D_OUT], f32,
                           kind="ExternalOutput").ap()

    relu = mybir.ActivationFunctionType.Relu

    with tile.TileContext(nc) as tc, ExitStack() as ctx:
        xp = ctx.enter_context(tc.tile_pool(name="x", bufs=2))
        w1p = ctx.enter_context(tc.tile_pool(name="w1", bufs=4))
        w2p = ctx.enter_context(tc.tile_pool(name="w2", bufs=2))
        hp = ctx.enter_context(tc.tile_pool(name="hid", bufs=2))
        op = ctx.enter_context(tc.tile_pool(name="o", bufs=4))
        ps1 = ctx.enter_context(tc.tile_pool(name="ps1", bufs=4, space="PSUM"))
        ps2 = ctx.enter_context(tc.tile_pool(name="ps2", bufs=2, space="PSUM"))

        for e in range(E_PER):
            # Load order on the sync HWDGE ring (FIFO per engine): the first
            # two w1 h-tiles ahead of the x stream so GEMM1 h0 can start
            # immediately; remaining w1 tiles + w2 after x.
            w1_tiles = []
            for h in range(2):
                w1_sb = w1p.tile([P, DT * P], bf16, tag="w1")
                nc.sync.dma_start(w1_sb[:], w1_d[e, h])
                w1_tiles.append(w1_sb)
            # xT for this expert: [128, 24, 1024], one DMA per d-tile so
            # GEMM1 can start before the whole expert is resident.
            x_sb = xp.tile([P, DT, N], bf16, tag="x")
            for d in range(DT):
                nc.sync.dma_start(x_sb[:, d, :], x_d[e, :, d, :])
            for h in range(2, HT):
                w1_sb = w1p.tile([P, DT * P], bf16, tag="w1")
                nc.sync.dma_start(w1_sb[:], w1_d[e, h])
                w1_tiles.append(w1_sb)
            w2_sb = w2p.tile([P, HT, D_OUT], bf16, tag="w2")
            nc.sync.dma_start(w2_sb[:], w2_d[e])

            hid = hp.tile([P, HT, N], bf16, tag="hid")

            # GEMM1 + relu -> hiddenT (bf16)
            for h in range(HT):
                w1_sb = w1_tiles[h]
                pa = ps1.tile([P, FD], f32, tag="ps1")
                pb = ps1.tile([P, FD], f32, tag="ps1")
                for d in range(DT):
                    lhsT = w1_sb[:, bass.ts(d, P)]
                    nc.tensor.matmul(pa[:], lhsT, x_sb[:, d, 0:FD],
                                     start=(d == 0), stop=(d == DT - 1))
                    nc.tensor.matmul(pb[:], lhsT, x_sb[:, d, FD:N],
                                     start=(d == 0), stop=(d == DT - 1))
                nc.scalar.activation(hid[:, h, 0:FD], pa[:], relu)
                nc.scalar.activation(hid[:, h, FD:N], pb[:], relu)

            # GEMM2 + relu -> out rows
            for nt in range(NT):
                po = ps2.tile([P, D_OUT], f32, tag="ps2")
                for k in range(HT):
                    nc.tensor.matmul(po[:], hid[:, k, bass.ts(nt, P)],
                                     w2_sb[:, k, :],
                                     start=(k == 0), stop=(k == HT - 1))
                o_sb = op.tile([P, D_OUT], f32, tag="o")
                nc.scalar.activation(o_sb[:], po[:], relu)
                # Output stores on the gpsimd (SWDGE) queue so their waits
                # don't head-of-line-block the next expert's loads on sync.
                nc.gpsimd.dma_start(out_d[e, bass.ds(nt * P, P), :], o_sb[:])

    nc.compile()
    _CACHE["nc"] = nc
    _CACHE["names"] = ("xt", "w1t", "w2t", "out")
    return nc, _CACHE["names"]


def _prep_inputs(x: np.ndarray, w1: np.ndarray, w2: np.ndarray):
    """Shard across cores + cast bf16 + pre-tile so all DMAs are contiguous."""
    # xT, partition-major: xt[e, p, d, n] = x[e, n, d*128+p]
    xt = np.ascontiguousarray(
        x.astype(_BF16).transpose(0, 2, 1)      # [E, D_IN, N]
        .reshape(E, DT, P, N).transpose(0, 2, 1, 3))  # [E, P, DT, N]
    # w1 h-tiled, partition-major: w1t[e, h, p, dt*128+c] = w1[e, dt*128+p, h*128+c]
    w1t = np.ascontiguousarray(
        w1.astype(_BF16).reshape(E, DT, P, HT, P)
        .transpose(0, 3, 2, 1, 4).reshape(E, HT, P, DT * P))
    # w2 k-tiled, partition-major: w2t[e, p, k, o] = w2[e, k*128+p, o]
    w2t = np.ascontiguousarray(
        w2.astype(_BF16).reshape(E, HT, P, D_OUT).transpose(0, 2, 1, 3))

    in_maps = []
    for c in range(NCORES):
        sl = slice(c * E_PER, (c + 1) * E_PER)
        in_maps.append({"xt": xt[sl], "w1t": w1t[sl], "w2t": w2t[sl]})
    return in_maps


def run(x, w1, w2, trace=False, **trace_kwargs):
    """Run on 8 cores; returns (full_out, BassKernelResults)."""
    from concourse.bass_utils import run_bass_kernel_spmd

    nc, _ = _build_program()
    in_maps = _prep_inputs(np.asarray(x), np.asarray(w1), np.asarray(w2))
    res = run_bass_kernel_spmd(nc, in_maps, list(range(NCORES)), trace=trace,
                               **trace_kwargs)
    out = np.concatenate([res.results[c]["out"] for c in range(NCORES)], axis=0)
    return out, res


def kernel(x: np.ndarray, w1: np.ndarray, w2: np.ndarray) -> np.ndarray:
    out, _ = run(x, w1, w2, trace=False)
    return out


# revision 5
# speedup vs baseline: 1.0399x; 1.0070x over previous
"""Trainium2 Bass kernel for per-expert MLP (MoE experts, expert-parallel).

Computes out = relu(relu(x @ w1) @ w2) per expert.
  x:  [E=32, N=1024, D_IN=3072] f32
  w1: [E, D_IN, D_H=1024] f32
  w2: [E, D_H, D_OUT=256] f32
  out:[E, N, D_OUT] f32

Sharding: expert dim E=32 split across 8 cores (4 experts/core), no
communication. Host pre-casts to bf16 and pre-tiles layouts so every DMA is
a plain partition-major copy and no on-chip transposes are needed:

GEMM1 computes hiddenT (h on partitions) directly:
  hiddenT[h, n] = sum_d w1[d, h] * x[n, d]
  lhsT = w1 tile [d(128 part), h(128 cols)]   (stationary)
  rhs  = xT tile [d(128 part), n(512 free)]   (moving)
GEMM2 then has contraction dim h already on partitions:
  out[n, o] = sum_h hiddenT[h, n] * w2[h, o]
  lhsT = hiddenT tile [h(128), n(128)], rhs = w2 tile [h(128), o(256)]
"""

import numpy as np
import ml_dtypes

E, N, D_IN, D_H, D_OUT = 32, 1024, 3072, 1024, 256
NCORES = 8
E_PER = E // NCORES  # 4 experts per core
P = 128
DT = D_IN // P  # 24 k-tiles for GEMM1
HT = D_H // P   # 8 h-tiles
NT = N // P     # 8 n-tiles
FD = 512        # matmul free dim (one PSUM bank of f32)
NCH = N // FD   # 2 n-chunks in GEMM1

_BF16 = ml_dtypes.bfloat16
_CACHE = {}


def _build_program():
    """Build + compile the per-core Bass program (same program on all cores)."""
    if "nc" in _CACHE:
        return _CACHE["nc"], _CACHE["names"]

    from contextlib import ExitStack

    import concourse.bass as bass
    import concourse.tile as tile
    from concourse import bacc, mybir

    bf16 = mybir.dt.bfloat16
    f32 = mybir.dt.float32

    nc = bacc.Bacc("TRN2", target_bir_lowering=False, debug=False,
                   enable_asserts=False)

    # Per-core DRAM I/O (host-prepped layouts, see kernel() below).
    x_d = nc.dram_tensor("xt", [E_PER, P, DT, N], bf16, kind="ExternalInput").ap()
    w1_d = nc.dram_tensor("w1t", [E_PER, HT, P, DT * P], bf16,
                          kind="ExternalInput").ap()
    w2_d = nc.dram_tensor("w2t", [E_PER, P, HT, D_OUT], bf16,
                          kind="ExternalInput").ap()
    out_d = nc.dram_tensor("out", [E_PER, N, D_OUT], f32,
                           kind="ExternalOutput").ap()

    relu = mybir.ActivationFunctionType.Relu

    with tile.TileContext(nc) as tc, ExitStack() as ctx:
        xp = ctx.enter_context(tc.tile_pool(name="x", bufs=2))
        w1p = ctx.enter_context(tc.tile_pool(name="w1", bufs=4))
        w2p = ctx.enter_context(tc.tile_pool(name="w2", bufs=2))
        hp = ctx.enter_context(tc.tile_pool(name="hid", bufs=2))
        op = ctx.enter_context(tc.tile_pool(name="o", bufs=6))
        wmp = ctx.enter_context(tc.tile_pool(name="warm", bufs=1))
        ps1 = ctx.enter_context(tc.tile_pool(name="ps1", bufs=4, space="PSUM"))
        ps2 = ctx.enter_context(tc.tile_pool(name="ps2", bufs=2, space="PSUM"))
        psw = ctx.enter_context(tc.tile_pool(name="psw", bufs=1, space="PSUM"))

        # PE warm-up: dummy matmuls with no data deps fill the initial DMA
        # wait so the HAM clock-gate is at 8/8 (2.4 GHz) when real matmuls
        # start (the un-throttle needs ~3.4us of sustained PE activity).
        warm = wmp.tile([P, FD], bf16, tag="warm")
        nc.vector.memset(warm[:], 0.0)
        pw = psw.tile([P, FD], f32, tag="psw")
        for _ in range(32):
            nc.tensor.matmul(pw[:], warm[:, 0:P], warm[:], start=True, stop=True)

        for e in range(E_PER):
            # Load order on the sync HWDGE ring (FIFO per engine): w1 h0 and
            # the first x d-tile ahead of everything so GEMM1 h0 can start
            # immediately; remaining w1 tiles + w2 after the x stream.
            w1_tiles = []
            x_sb = xp.tile([P, DT, N], bf16, tag="x")
            w1_sb = w1p.tile([P, DT * P], bf16, tag="w1")
            nc.sync.dma_start(w1_sb[:], w1_d[e, 0])
            w1_tiles.append(w1_sb)
            nc.sync.dma_start(x_sb[:, 0, :], x_d[e, :, 0, :])
            w1_sb = w1p.tile([P, DT * P], bf16, tag="w1")
            nc.sync.dma_start(w1_sb[:], w1_d[e, 1])
            w1_tiles.append(w1_sb)
            for d in range(1, DT):
                nc.sync.dma_start(x_sb[:, d, :], x_d[e, :, d, :])
            for h in range(2, HT):
                w1_sb = w1p.tile([P, DT * P], bf16, tag="w1")
                nc.sync.dma_start(w1_sb[:], w1_d[e, h])
                w1_tiles.append(w1_sb)
            w2_sb = w2p.tile([P, HT, D_OUT], bf16, tag="w2")
            nc.sync.dma_start(w2_sb[:], w2_d[e])

            hid = hp.tile([P, HT, N], bf16, tag="hid")

            # GEMM1 + relu -> hiddenT (bf16)
            for h in range(HT):
                w1_sb = w1_tiles[h]
                pa = ps1.tile([P, FD], f32, tag="ps1")
                pb = ps1.tile([P, FD], f32, tag="ps1")
                for d in range(DT):
                    lhsT = w1_sb[:, bass.ts(d, P)]
                    nc.tensor.matmul(pa[:], lhsT, x_sb[:, d, 0:FD],
                                     start=(d == 0), stop=(d == DT - 1))
                    nc.tensor.matmul(pb[:], lhsT, x_sb[:, d, FD:N],
                                     start=(d == 0), stop=(d == DT - 1))
                nc.scalar.activation(hid[:, h, 0:FD], pa[:], relu)
                nc.scalar.activation(hid[:, h, FD:N], pb[:], relu)

            # GEMM2 + relu -> out rows
            for nt in range(NT):
                po = ps2.tile([P, D_OUT], f32, tag="ps2")
                for k in range(HT):
                    nc.tensor.matmul(po[:], hid[:, k, bass.ts(nt, P)],
                                     w2_sb[:, k, :],
                                     start=(k == 0), stop=(k == HT - 1))
                o_sb = op.tile([P, D_OUT], f32, tag="o")
                nc.scalar.activation(o_sb[:], po[:], relu)
                # Output stores on the scalar HWDGE ring: same engine as the
                # relu (ordering is free) and they don't head-of-line-block
                # the next expert's loads on the sync ring.
                nc.scalar.dma_start(out_d[e, bass.ds(nt * P, P), :], o_sb[:])

    nc.compile()
    _CACHE["nc"] = nc
    _CACHE["names"] = ("xt", "w1t", "w2t", "out")
    return nc, _CACHE["names"]


def _prep_inputs(x: np.ndarray, w1: np.ndarray, w2: np.ndarray):
    """Shard across cores + cast bf16 + pre-tile so all DMAs are contiguous."""
    # xT, partition-major: xt[e, p, d, n] = x[e, n, d*128+p]
    xt = np.ascontiguousarray(
        x.astype(_BF16).transpose(0, 2, 1)      # [E, D_IN, N]
        .reshape(E, DT, P, N).transpose(0, 2, 1, 3))  # [E, P, DT, N]
    # w1 h-tiled, partition-major: w1t[e, h, p, dt*128+c] = w1[e, dt*128+p, h*128+c]
    w1t = np.ascontiguousarray(
        w1.astype(_BF16).reshape(E, DT, P, HT, P)
        .transpose(0, 3, 2, 1, 4).reshape(E, HT, P, DT * P))
    # w2 k-tiled, partition-major: w2t[e, p, k, o] = w2[e, k*128+p, o]
    w2t = np.ascontiguousarray(
        w2.astype(_BF16).reshape(E, HT, P, D_OUT).transpose(0, 2, 1, 3))

    in_maps = []
    for c in range(NCORES):
        sl = slice(c * E_PER, (c + 1) * E_PER)
        in_maps.append({"xt": xt[sl], "w1t": w1t[sl], "w2t": w2t[sl]})
    return in_maps


def run(x, w1, w2, trace=False, **trace_kwargs):
    """Run on 8 cores; returns (full_out, BassKernelResults)."""
    from concourse.bass_utils import run_bass_kernel_spmd

    nc, _ = _build_program()
    in_maps = _prep_inputs(np.asarray(x), np.asarray(w1), np.asarray(w2))
    res = run_bass_kernel_spmd(nc, in_maps, list(range(NCORES)), trace=trace,
                               **trace_kwargs)
    out = np.concatenate([res.results[c]["out"] for c in range(NCORES)], axis=0)
    return out, res


def kernel(x: np.ndarray, w1: np.ndarray, w2: np.ndarray) -> np.ndarray:
    out, _ = run(x, w1, w2, trace=False)
    return out


# revision 8
# speedup vs baseline: 1.0532x; 1.0128x over previous
"""Trainium2 Bass kernel for per-expert MLP (MoE experts, expert-parallel).

Computes out = relu(relu(x @ w1) @ w2) per expert.
  x:  [E=32, N=1024, D_IN=3072] f32
  w1: [E, D_IN, D_H=1024] f32
  w2: [E, D_H, D_OUT=256] f32
  out:[E, N, D_OUT] f32

Sharding: expert dim E=32 split across 8 cores (4 experts/core), no
communication. Host pre-casts to bf16 and pre-tiles layouts so every DMA is
a plain partition-major copy and no on-chip transposes are needed:

GEMM1 computes hiddenT (h on partitions) directly:
  hiddenT[h, n] = sum_d w1[d, h] * x[n, d]
  lhsT = w1 tile [d(128 part), h(128 cols)]   (stationary)
  rhs  = xT tile [d(128 part), n(512 free)]   (moving)
GEMM2 then has contraction dim h already on partitions:
  out[n, o] = sum_h hiddenT[h, n] * w2[h, o]
  lhsT = hiddenT tile [h(128), n(128)], rhs = w2 tile [h(128), o(256)]
"""

import numpy as np
import ml_dtypes

E, N, D_IN, D_H, D_OUT = 32, 1024, 3072, 1024, 256
NCORES = 8
E_PER = E // NCORES  # 4 experts per core
P = 128
DT = D_IN // P  # 24 k-tiles for GEMM1
HT = D_H // P   # 8 h-tiles
NT = N // P     # 8 n-tiles
FD = 512        # matmul free dim (one PSUM bank of f32)
NCH = N // FD   # 2 n-chunks in GEMM1

_BF16 = ml_dtypes.bfloat16
_CACHE = {}


def _build_program():
    """Build + compile the per-core Bass program (same program on all cores)."""
    if "nc" in _CACHE:
        return _CACHE["nc"], _CACHE["names"]

    from contextlib import ExitStack

    import concourse.bass as bass
    import concourse.tile as tile
    from concourse import bacc, mybir

    bf16 = mybir.dt.bfloat16
    f32 = mybir.dt.float32

    nc = bacc.Bacc("TRN2", target_bir_lowering=False, debug=False,
                   enable_asserts=False)

    # Per-core DRAM I/O (host-prepped layouts, see kernel() below).
    x_d = nc.dram_tensor("xt", [E_PER, P, DT, N], bf16, kind="ExternalInput").ap()
    w1_d = nc.dram_tensor("w1t", [E_PER, HT, P, DT * P], bf16,
                          kind="ExternalInput").ap()
    w2_d = nc.dram_tensor("w2t", [E_PER, P, HT, D_OUT], bf16,
                          kind="ExternalInput").ap()
    out_d = nc.dram_tensor("out", [E_PER, N, D_OUT], f32,
                           kind="ExternalOutput").ap()

    relu = mybir.ActivationFunctionType.Relu

    with tile.TileContext(nc) as tc, ExitStack() as ctx:
        xp = ctx.enter_context(tc.tile_pool(name="x", bufs=2))
        w1p = ctx.enter_context(tc.tile_pool(name="w1", bufs=4))
        w2p = ctx.enter_context(tc.tile_pool(name="w2", bufs=2))
        hp = ctx.enter_context(tc.tile_pool(name="hid", bufs=2))
        op = ctx.enter_context(tc.tile_pool(name="o", bufs=2))
        wmp = ctx.enter_context(tc.tile_pool(name="warm", bufs=1))
        ps1 = ctx.enter_context(tc.tile_pool(name="ps1", bufs=4, space="PSUM"))
        ps2 = ctx.enter_context(tc.tile_pool(name="ps2", bufs=2, space="PSUM"))
        psw = ctx.enter_context(tc.tile_pool(name="psw", bufs=1, space="PSUM"))

        # PE warm-up: dummy matmuls with no data deps fill the initial DMA
        # wait so the HAM clock-gate is at 8/8 (2.4 GHz) when real matmuls
        # start (the un-throttle needs ~3.4us of sustained PE activity).
        warm = wmp.tile([P, FD], bf16, tag="warm")
        nc.vector.memset(warm[:], 0.0)
        pw = psw.tile([P, FD], f32, tag="psw")
        for _ in range(32):
            nc.tensor.matmul(pw[:], warm[:, 0:P], warm[:], start=True, stop=True)

        for e in range(E_PER):
            # Load order on the sync HWDGE ring (FIFO per engine): w1 h0 and
            # the first x d-tile ahead of everything so GEMM1 h0 can start
            # immediately; remaining w1 tiles + w2 after the x stream.
            w1_tiles = []
            x_sb = xp.tile([P, DT, N], bf16, tag="x")
            w1_sb = w1p.tile([P, DT * P], bf16, tag="w1")
            nc.sync.dma_start(w1_sb[:], w1_d[e, 0])
            w1_tiles.append(w1_sb)
            nc.sync.dma_start(x_sb[:, 0, :], x_d[e, :, 0, :])
            w1_sb = w1p.tile([P, DT * P], bf16, tag="w1")
            nc.sync.dma_start(w1_sb[:], w1_d[e, 1])
            w1_tiles.append(w1_sb)
            if e == 0:
                # fine-grained so first-expert GEMM1 is paced per d-tile
                for d in range(1, DT):
                    nc.sync.dma_start(x_sb[:, d, :], x_d[e, :, d, :])
            else:
                # prefetched during previous expert: coarse chunks to limit
                # HWDGE sem-lane churn (8 lanes shared across all queues)
                for d in range(1, 5):
                    nc.sync.dma_start(x_sb[:, d, :], x_d[e, :, d, :])
                for i in range(5, DT, 4):
                    j = min(i + 4, DT)
                    nc.sync.dma_start(x_sb[:, i:j, :], x_d[e, :, i:j, :])
            for h in range(2, HT):
                w1_sb = w1p.tile([P, DT * P], bf16, tag="w1")
                nc.sync.dma_start(w1_sb[:], w1_d[e, h])
                w1_tiles.append(w1_sb)
            w2_sb = w2p.tile([P, HT, D_OUT], bf16, tag="w2")
            nc.sync.dma_start(w2_sb[:], w2_d[e])

            hid = hp.tile([P, HT, N], bf16, tag="hid")

            # GEMM1 + relu -> hiddenT (bf16)
            for h in range(HT):
                w1_sb = w1_tiles[h]
                pa = ps1.tile([P, FD], f32, tag="ps1")
                pb = ps1.tile([P, FD], f32, tag="ps1")
                for d in range(DT):
                    lhsT = w1_sb[:, bass.ts(d, P)]
                    nc.tensor.matmul(pa[:], lhsT, x_sb[:, d, 0:FD],
                                     start=(d == 0), stop=(d == DT - 1))
                    nc.tensor.matmul(pb[:], lhsT, x_sb[:, d, FD:N],
                                     start=(d == 0), stop=(d == DT - 1))
                nc.scalar.activation(hid[:, h, 0:FD], pa[:], relu)
                nc.scalar.activation(hid[:, h, FD:N], pb[:], relu)

            # GEMM2 + relu -> out rows, accumulated in SBUF and stored with a
            # single 1MB DMA per expert (avoids per-n-tile store DMAs whose
            # HWDGE sem-lane reuse couples to in-flight prefetch loads and
            # stalls the relu/psum pipeline mid-GEMM2).
            o_sb = op.tile([P, NT, D_OUT], f32, tag="o")
            for nt in range(NT):
                po = ps2.tile([P, D_OUT], f32, tag="ps2")
                for k in range(HT):
                    nc.tensor.matmul(po[:], hid[:, k, bass.ts(nt, P)],
                                     w2_sb[:, k, :],
                                     start=(k == 0), stop=(k == HT - 1))
                nc.scalar.activation(o_sb[:, nt, :], po[:], relu)
            nc.scalar.dma_start(
                out_d[e].rearrange("(nt p) o -> p nt o", p=P), o_sb[:])

    nc.compile()
    _CACHE["nc"] = nc
    _CACHE["names"] = ("xt", "w1t", "w2t", "out")
    return nc, _CACHE["names"]


def _prep_inputs(x: np.ndarray, w1: np.ndarray, w2: np.ndarray):
    """Shard across cores + cast bf16 + pre-tile so all DMAs are contiguous."""
    # xT, partition-major: xt[e, p, d, n] = x[e, n, d*128+p]
    xt = np.ascontiguousarray(
        x.astype(_BF16).transpose(0, 2, 1)      # [E, D_IN, N]
        .reshape(E, DT, P, N).transpose(0, 2, 1, 3))  # [E, P, DT, N]
    # w1 h-tiled, partition-major: w1t[e, h, p, dt*128+c] = w1[e, dt*128+p, h*128+c]
    w1t = np.ascontiguousarray(
        w1.astype(_BF16).reshape(E, DT, P, HT, P)
        .transpose(0, 3, 2, 1, 4).reshape(E, HT, P, DT * P))
    # w2 k-tiled, partition-major: w2t[e, p, k, o] = w2[e, k*128+p, o]
    w2t = np.ascontiguousarray(
        w2.astype(_BF16).reshape(E, HT, P, D_OUT).transpose(0, 2, 1, 3))

    in_maps = []
    for c in range(NCORES):
        sl = slice(c * E_PER, (c + 1) * E_PER)
        in_maps.append({"xt": xt[sl], "w1t": w1t[sl], "w2t": w2t[sl]})
    return in_maps


def run(x, w1, w2, trace=False, **trace_kwargs):
    """Run on 8 cores; returns (full_out, BassKernelResults)."""
    from concourse.bass_utils import run_bass_kernel_spmd

    nc, _ = _build_program()
    in_maps = _prep_inputs(np.asarray(x), np.asarray(w1), np.asarray(w2))
    res = run_bass_kernel_spmd(nc, in_maps, list(range(NCORES)), trace=trace,
                               **trace_kwargs)
    out = np.concatenate([res.results[c]["out"] for c in range(NCORES)], axis=0)
    return out, res


def kernel(x: np.ndarray, w1: np.ndarray, w2: np.ndarray) -> np.ndarray:
    out, _ = run(x, w1, w2, trace=False)
    return out


# revision 11
# speedup vs baseline: 1.0545x; 1.0012x over previous
"""Trainium2 Bass kernel for per-expert MLP (MoE experts, expert-parallel).

Computes out = relu(relu(x @ w1) @ w2) per expert.
  x:  [E=32, N=1024, D_IN=3072] f32
  w1: [E, D_IN, D_H=1024] f32
  w2: [E, D_H, D_OUT=256] f32
  out:[E, N, D_OUT] f32

Sharding: expert dim E=32 split across 8 cores (4 experts/core), no
communication. Host pre-casts to bf16 and pre-tiles layouts so every DMA is
a plain partition-major copy and no on-chip transposes are needed:

GEMM1 computes hiddenT (h on partitions) directly:
  hiddenT[h, n] = sum_d w1[d, h] * x[n, d]
  lhsT = w1 tile [d(128 part), h(128 cols)]   (stationary)
  rhs  = xT tile [d(128 part), n(512 free)]   (moving)
GEMM2 then has contraction dim h already on partitions:
  out[n, o] = sum_h hiddenT[h, n] * w2[h, o]
  lhsT = hiddenT tile [h(128), n(128)], rhs = w2 tile [h(128), o(256)]
"""

import numpy as np
import ml_dtypes

E, N, D_IN, D_H, D_OUT = 32, 1024, 3072, 1024, 256
NCORES = 8
E_PER = E // NCORES  # 4 experts per core
P = 128
DT = D_IN // P  # 24 k-tiles for GEMM1
HT = D_H // P   # 8 h-tiles
NT = N // P     # 8 n-tiles
FD = 512        # matmul free dim (one PSUM bank of f32)
NCH = N // FD   # 2 n-chunks in GEMM1

_BF16 = ml_dtypes.bfloat16
_CACHE = {}


def _build_program():
    """Build + compile the per-core Bass program (same program on all cores)."""
    if "nc" in _CACHE:
        return _CACHE["nc"], _CACHE["names"]

    from contextlib import ExitStack

    import concourse.bass as bass
    import concourse.tile as tile
    from concourse import bacc, mybir

    bf16 = mybir.dt.bfloat16
    f32 = mybir.dt.float32

    nc = bacc.Bacc("TRN2", target_bir_lowering=False, debug=False,
                   enable_asserts=False)

    # Per-core DRAM I/O (host-prepped layouts, see kernel() below).
    x_d = nc.dram_tensor("xt", [E_PER, P, DT, N], bf16, kind="ExternalInput").ap()
    w1_d = nc.dram_tensor("w1t", [E_PER, HT, P, DT * P], bf16,
                          kind="ExternalInput").ap()
    w2_d = nc.dram_tensor("w2t", [E_PER, P, HT, D_OUT], bf16,
                          kind="ExternalInput").ap()
    out_d = nc.dram_tensor("out", [E_PER, N, D_OUT], f32,
                           kind="ExternalOutput").ap()

    relu = mybir.ActivationFunctionType.Relu

    with tile.TileContext(nc) as tc, ExitStack() as ctx:
        xp = ctx.enter_context(tc.tile_pool(name="x", bufs=2))
        w1p = ctx.enter_context(tc.tile_pool(name="w1", bufs=4))
        w2p = ctx.enter_context(tc.tile_pool(name="w2", bufs=2))
        hp = ctx.enter_context(tc.tile_pool(name="hid", bufs=2))
        op = ctx.enter_context(tc.tile_pool(name="o", bufs=2))
        wmp = ctx.enter_context(tc.tile_pool(name="warm", bufs=1))
        ps1 = ctx.enter_context(tc.tile_pool(name="ps1", bufs=4, space="PSUM"))
        ps2 = ctx.enter_context(tc.tile_pool(name="ps2", bufs=2, space="PSUM"))
        psw = ctx.enter_context(tc.tile_pool(name="psw", bufs=1, space="PSUM"))

        # PE warm-up: dummy matmuls with no data deps fill the initial DMA
        # wait so the HAM clock-gate is at 8/8 (2.4 GHz) when real matmuls
        # start (the un-throttle needs ~3.4us of sustained PE activity).
        # One accumulation group: no per-matmul PSUM WAW serialization.
        NWARM = 18
        warm = wmp.tile([P, FD], bf16, tag="warm")
        nc.vector.memset(warm[:], 0.0)
        pw = psw.tile([P, FD], f32, tag="psw")
        for i in range(NWARM):
            nc.tensor.matmul(pw[:], warm[:, 0:P], warm[:],
                             start=(i == 0), stop=(i == NWARM - 1))

        for e in range(E_PER):
            # Load order on the sync HWDGE ring (FIFO per engine): w1 h0 and
            # the first x d-tile ahead of everything so GEMM1 h0 can start
            # immediately; remaining w1 tiles + w2 after the x stream.
            w1_tiles = []
            x_sb = xp.tile([P, DT, N], bf16, tag="x")
            w1_sb = w1p.tile([P, DT * P], bf16, tag="w1")
            if e == 0:
                # Chunk the first w1 tile so GEMM1's first matmuls only wait
                # on a 256KB transfer, and pace x per d-tile: the whole e0
                # ramp is DMA-bandwidth-bound, so start compute ASAP.
                nc.sync.dma_start(w1_sb[:, 0: 8 * P], w1_d[e, 0, :, 0: 8 * P])
                w1_tiles.append(w1_sb)
                nc.sync.dma_start(x_sb[:, 0, :], x_d[e, :, 0, :])
                nc.sync.dma_start(x_sb[:, 1, :], x_d[e, :, 1, :])
                nc.sync.dma_start(w1_sb[:, 8 * P: 16 * P],
                                  w1_d[e, 0, :, 8 * P: 16 * P])
                nc.sync.dma_start(x_sb[:, 2, :], x_d[e, :, 2, :])
                nc.sync.dma_start(x_sb[:, 3, :], x_d[e, :, 3, :])
                nc.sync.dma_start(w1_sb[:, 16 * P: DT * P],
                                  w1_d[e, 0, :, 16 * P: DT * P])
                for d in range(4, 8):
                    nc.sync.dma_start(x_sb[:, d, :], x_d[e, :, d, :])
                w1_sb = w1p.tile([P, DT * P], bf16, tag="w1")
                nc.sync.dma_start(w1_sb[:], w1_d[e, 1])
                w1_tiles.append(w1_sb)
                for d in range(8, DT):
                    nc.sync.dma_start(x_sb[:, d, :], x_d[e, :, d, :])
            else:
                # prefetched during previous expert: coarse chunks to limit
                # HWDGE sem-lane churn (8 lanes shared across all queues)
                nc.sync.dma_start(w1_sb[:], w1_d[e, 0])
                w1_tiles.append(w1_sb)
                nc.sync.dma_start(x_sb[:, 0, :], x_d[e, :, 0, :])
                w1_sb = w1p.tile([P, DT * P], bf16, tag="w1")
                nc.sync.dma_start(w1_sb[:], w1_d[e, 1])
                w1_tiles.append(w1_sb)
                for d in range(1, 5):
                    nc.sync.dma_start(x_sb[:, d, :], x_d[e, :, d, :])
                for i in range(5, DT, 4):
                    j = min(i + 4, DT)
                    nc.sync.dma_start(x_sb[:, i:j, :], x_d[e, :, i:j, :])
            for h in range(2, HT):
                w1_sb = w1p.tile([P, DT * P], bf16, tag="w1")
                nc.sync.dma_start(w1_sb[:], w1_d[e, h])
                w1_tiles.append(w1_sb)
            w2_sb = w2p.tile([P, HT, D_OUT], bf16, tag="w2")
            nc.sync.dma_start(w2_sb[:], w2_d[e])

            hid = hp.tile([P, HT, N], bf16, tag="hid")

            # GEMM1 + relu -> hiddenT (bf16)
            for h in range(HT):
                w1_sb = w1_tiles[h]
                pa = ps1.tile([P, FD], f32, tag="ps1")
                pb = ps1.tile([P, FD], f32, tag="ps1")
                for d in range(DT):
                    lhsT = w1_sb[:, bass.ts(d, P)]
                    nc.tensor.matmul(pa[:], lhsT, x_sb[:, d, 0:FD],
                                     start=(d == 0), stop=(d == DT - 1))
                    nc.tensor.matmul(pb[:], lhsT, x_sb[:, d, FD:N],
                                     start=(d == 0), stop=(d == DT - 1))
                nc.scalar.activation(hid[:, h, 0:FD], pa[:], relu)
                nc.scalar.activation(hid[:, h, FD:N], pb[:], relu)

            # GEMM2 + relu -> out rows, accumulated in SBUF and stored with a
            # single 1MB DMA per expert (avoids per-n-tile store DMAs whose
            # HWDGE sem-lane reuse couples to in-flight prefetch loads and
            # stalls the relu/psum pipeline mid-GEMM2).
            o_sb = op.tile([P, NT, D_OUT], f32, tag="o")
            last_e = e == E_PER - 1
            for nt in range(NT):
                po = ps2.tile([P, D_OUT], f32, tag="ps2")
                for k in range(HT):
                    nc.tensor.matmul(po[:], hid[:, k, bass.ts(nt, P)],
                                     w2_sb[:, k, :],
                                     start=(k == 0), stop=(k == HT - 1))
                nc.scalar.activation(o_sb[:, nt, :], po[:], relu)
                if last_e:
                    # tail: store per n-tile so the final store isn't one
                    # serialized 1MB DMA after the last relu (no loads are in
                    # flight now, so sem-lane reuse can't stall anything)
                    nc.scalar.dma_start(out_d[e, bass.ds(nt * P, P), :],
                                        o_sb[:, nt, :])
            if not last_e:
                nc.scalar.dma_start(
                    out_d[e].rearrange("(nt p) o -> p nt o", p=P), o_sb[:])

    nc.compile()
    _CACHE["nc"] = nc
    _CACHE["names"] = ("xt", "w1t", "w2t", "out")
    return nc, _CACHE["names"]


def _prep_inputs(x: np.ndarray, w1: np.ndarray, w2: np.ndarray):
    """Shard across cores + cast bf16 + pre-tile so all DMAs are contiguous."""
    # xT, partition-major: xt[e, p, d, n] = x[e, n, d*128+p]
    xt = np.ascontiguousarray(
        x.astype(_BF16).transpose(0, 2, 1)      # [E, D_IN, N]
        .reshape(E, DT, P, N).transpose(0, 2, 1, 3))  # [E, P, DT, N]
    # w1 h-tiled, partition-major: w1t[e, h, p, dt*128+c] = w1[e, dt*128+p, h*128+c]
    w1t = np.ascontiguousarray(
        w1.astype(_BF16).reshape(E, DT, P, HT, P)
        .transpose(0, 3, 2, 1, 4).reshape(E, HT, P, DT * P))
    # w2 k-tiled, partition-major: w2t[e, p, k, o] = w2[e, k*128+p, o]
    w2t = np.ascontiguousarray(
        w2.astype(_BF16).reshape(E, HT, P, D_OUT).transpose(0, 2, 1, 3))

    in_maps = []
    for c in range(NCORES):
        sl = slice(c * E_PER, (c + 1) * E_PER)
        in_maps.append({"xt": xt[sl], "w1t": w1t[sl], "w2t": w2t[sl]})
    return in_maps


def run(x, w1, w2, trace=False, **trace_kwargs):
    """Run on 8 cores; returns (full_out, BassKernelResults)."""
    from concourse.bass_utils import run_bass_kernel_spmd

    nc, _ = _build_program()
    in_maps = _prep_inputs(np.asarray(x), np.asarray(w1), np.asarray(w2))
    res = run_bass_kernel_spmd(nc, in_maps, list(range(NCORES)), trace=trace,
                               **trace_kwargs)
    out = np.concatenate([res.results[c]["out"] for c in range(NCORES)], axis=0)
    return out, res


def kernel(x: np.ndarray, w1: np.ndarray, w2: np.ndarray) -> np.ndarray:
    out, _ = run(x, w1, w2, trace=False)
    return out


# revision 14
# speedup vs baseline: 1.0663x; 1.0113x over previous
"""Trainium2 Bass kernel for per-expert MLP (MoE experts, expert-parallel).

Computes out = relu(relu(x @ w1) @ w2) per expert.
  x:  [E=32, N=1024, D_IN=3072] f32
  w1: [E, D_IN, D_H=1024] f32
  w2: [E, D_H, D_OUT=256] f32
  out:[E, N, D_OUT] f32

Sharding: expert dim E=32 split across 8 cores (4 experts/core), no
communication. Host pre-casts to bf16 and pre-tiles layouts so every DMA is
a plain partition-major copy and no on-chip transposes are needed:

GEMM1 computes hiddenT (h on partitions) directly:
  hiddenT[h, n] = sum_d w1[d, h] * x[n, d]
  lhsT = w1 tile [d(128 part), h(128 cols)]   (stationary)
  rhs  = xT tile [d(128 part), n(512 free)]   (moving)
GEMM2 then has contraction dim h already on partitions:
  out[n, o] = sum_h hiddenT[h, n] * w2[h, o]
  lhsT = hiddenT tile [h(128), n(128)], rhs = w2 tile [h(128), o(256)]
"""

import numpy as np
import ml_dtypes

E, N, D_IN, D_H, D_OUT = 32, 1024, 3072, 1024, 256
NCORES = 8
E_PER = E // NCORES  # 4 experts per core
P = 128
DT = D_IN // P  # 24 k-tiles for GEMM1
HT = D_H // P   # 8 h-tiles
NT = N // P     # 8 n-tiles
FD = 512        # matmul free dim (one PSUM bank of f32)
NCH = N // FD   # 2 n-chunks in GEMM1

_BF16 = ml_dtypes.bfloat16
_CACHE = {}


def _build_program():
    """Build + compile the per-core Bass program (same program on all cores)."""
    if "nc" in _CACHE:
        return _CACHE["nc"], _CACHE["names"]

    from contextlib import ExitStack

    import concourse.bass as bass
    import concourse.tile as tile
    from concourse import bacc, mybir

    bf16 = mybir.dt.bfloat16
    f32 = mybir.dt.float32

    nc = bacc.Bacc("TRN2", target_bir_lowering=False, debug=False,
                   enable_asserts=False)

    # Per-core DRAM I/O (host-prepped layouts, see kernel() below).
    x_d = nc.dram_tensor("xt", [E_PER, P, DT, N], bf16, kind="ExternalInput").ap()
    w1_d = nc.dram_tensor("w1t", [E_PER, HT, P, DT * P], bf16,
                          kind="ExternalInput").ap()
    w2_d = nc.dram_tensor("w2t", [E_PER, P, HT, D_OUT], bf16,
                          kind="ExternalInput").ap()
    out_d = nc.dram_tensor("out", [E_PER, N, D_OUT], f32,
                           kind="ExternalOutput").ap()

    relu = mybir.ActivationFunctionType.Relu

    with tile.TileContext(nc) as tc, ExitStack() as ctx:
        xp = ctx.enter_context(tc.tile_pool(name="x", bufs=2))
        w1p = ctx.enter_context(tc.tile_pool(name="w1", bufs=4))
        w2p = ctx.enter_context(tc.tile_pool(name="w2", bufs=2))
        hp = ctx.enter_context(tc.tile_pool(name="hid", bufs=2))
        op = ctx.enter_context(tc.tile_pool(name="o", bufs=2))
        wmp = ctx.enter_context(tc.tile_pool(name="warm", bufs=1))
        ps1 = ctx.enter_context(tc.tile_pool(name="ps1", bufs=4, space="PSUM"))
        ps2 = ctx.enter_context(tc.tile_pool(name="ps2", bufs=2, space="PSUM"))
        psw = ctx.enter_context(tc.tile_pool(name="psw", bufs=1, space="PSUM"))

        # PE warm-up: dummy matmuls with no data deps fill the initial DMA
        # wait so the HAM clock-gate is at 8/8 (2.4 GHz) when real matmuls
        # start (the un-throttle needs ~3.4us of sustained PE activity).
        # One accumulation group: no per-matmul PSUM WAW serialization.
        NWARM = 18
        warm = wmp.tile([P, FD], bf16, tag="warm")
        nc.vector.memset(warm[:], 0.0)
        pw = psw.tile([P, FD], f32, tag="psw")
        for i in range(NWARM):
            nc.tensor.matmul(pw[:], warm[:, 0:P], warm[:],
                             start=(i == 0), stop=(i == NWARM - 1))

        for e in range(E_PER):
            # Load order on the sync HWDGE ring (FIFO per engine): w1 h0 and
            # the first x d-tile ahead of everything so GEMM1 h0 can start
            # immediately; remaining w1 tiles + w2 after the x stream.
            w1_tiles = []
            x_sb = xp.tile([P, DT, N], bf16, tag="x")
            w1_sb = w1p.tile([P, DT * P], bf16, tag="w1")
            if e == 0:
                # Chunk the first w1 tile so GEMM1's first matmuls only wait
                # on a 256KB transfer, and pace x per d-tile: the whole e0
                # ramp is DMA-bandwidth-bound, so start compute ASAP.
                w1b_sb = w1p.tile([P, DT * P], bf16, tag="w1")
                nc.sync.dma_start(w1_sb[:, 0: 8 * P], w1_d[e, 0, :, 0: 8 * P])
                w1_tiles.append(w1_sb)
                nc.sync.dma_start(x_sb[:, 0, :], x_d[e, :, 0, :])
                nc.sync.dma_start(w1b_sb[:, 0: 8 * P], w1_d[e, 1, :, 0: 8 * P])
                nc.sync.dma_start(x_sb[:, 1, :], x_d[e, :, 1, :])
                nc.sync.dma_start(w1_sb[:, 8 * P: 16 * P],
                                  w1_d[e, 0, :, 8 * P: 16 * P])
                nc.sync.dma_start(w1b_sb[:, 8 * P: 16 * P],
                                  w1_d[e, 1, :, 8 * P: 16 * P])
                nc.sync.dma_start(x_sb[:, 2, :], x_d[e, :, 2, :])
                nc.sync.dma_start(x_sb[:, 3, :], x_d[e, :, 3, :])
                nc.sync.dma_start(w1_sb[:, 16 * P: DT * P],
                                  w1_d[e, 0, :, 16 * P: DT * P])
                nc.sync.dma_start(w1b_sb[:, 16 * P: DT * P],
                                  w1_d[e, 1, :, 16 * P: DT * P])
                w1_tiles.append(w1b_sb)
                for d in range(4, 8):
                    nc.sync.dma_start(x_sb[:, d, :], x_d[e, :, d, :])
                for d in range(8, DT):
                    nc.sync.dma_start(x_sb[:, d, :], x_d[e, :, d, :])
            else:
                # prefetched during previous expert: coarse chunks to limit
                # HWDGE sem-lane churn (8 lanes shared across all queues)
                nc.sync.dma_start(w1_sb[:], w1_d[e, 0])
                w1_tiles.append(w1_sb)
                nc.sync.dma_start(x_sb[:, 0, :], x_d[e, :, 0, :])
                w1_sb = w1p.tile([P, DT * P], bf16, tag="w1")
                nc.sync.dma_start(w1_sb[:], w1_d[e, 1])
                w1_tiles.append(w1_sb)
                for d in range(1, 5):
                    nc.sync.dma_start(x_sb[:, d, :], x_d[e, :, d, :])
                for i in range(5, DT, 4):
                    j = min(i + 4, DT)
                    nc.sync.dma_start(x_sb[:, i:j, :], x_d[e, :, i:j, :])
            for h in range(2, HT):
                w1_sb = w1p.tile([P, DT * P], bf16, tag="w1")
                nc.sync.dma_start(w1_sb[:], w1_d[e, h])
                w1_tiles.append(w1_sb)
            w2_sb = w2p.tile([P, HT, D_OUT], bf16, tag="w2")
            nc.sync.dma_start(w2_sb[:], w2_d[e])

            hid = hp.tile([P, HT, N], bf16, tag="hid")

            # GEMM1 + relu -> hiddenT (bf16). h0 and h1 are interleaved in
            # one d-pass: each arriving x d-tile feeds 4 matmuls, so the
            # DMA-paced first-expert ramp consumes x at ~arrival rate
            # instead of stalling h0 on the tail of the x stream.
            pa = [ps1.tile([P, FD], f32, tag="ps1", name=f"pa{i}")
                  for i in range(2)]
            pb = [ps1.tile([P, FD], f32, tag="ps1", name=f"pb{i}")
                  for i in range(2)]
            for d in range(DT):
                for hh in range(2):
                    lhsT = w1_tiles[hh][:, bass.ts(d, P)]
                    nc.tensor.matmul(pa[hh][:], lhsT, x_sb[:, d, 0:FD],
                                     start=(d == 0), stop=(d == DT - 1))
                    nc.tensor.matmul(pb[hh][:], lhsT, x_sb[:, d, FD:N],
                                     start=(d == 0), stop=(d == DT - 1))
            for hh in range(2):
                nc.scalar.activation(hid[:, hh, 0:FD], pa[hh][:], relu)
                nc.scalar.activation(hid[:, hh, FD:N], pb[hh][:], relu)
            for h in range(2, HT):
                w1_sb = w1_tiles[h]
                pa1 = ps1.tile([P, FD], f32, tag="ps1")
                pb1 = ps1.tile([P, FD], f32, tag="ps1")
                for d in range(DT):
                    lhsT = w1_sb[:, bass.ts(d, P)]
                    nc.tensor.matmul(pa1[:], lhsT, x_sb[:, d, 0:FD],
                                     start=(d == 0), stop=(d == DT - 1))
                    nc.tensor.matmul(pb1[:], lhsT, x_sb[:, d, FD:N],
                                     start=(d == 0), stop=(d == DT - 1))
                nc.scalar.activation(hid[:, h, 0:FD], pa1[:], relu)
                nc.scalar.activation(hid[:, h, FD:N], pb1[:], relu)

            # GEMM2 + relu -> out rows, accumulated in SBUF and stored with a
            # single 1MB DMA per expert (avoids per-n-tile store DMAs whose
            # HWDGE sem-lane reuse couples to in-flight prefetch loads and
            # stalls the relu/psum pipeline mid-GEMM2).
            o_sb = op.tile([P, NT, D_OUT], f32, tag="o")
            last_e = e == E_PER - 1
            for nt in range(NT):
                po = ps2.tile([P, D_OUT], f32, tag="ps2")
                for k in range(HT):
                    nc.tensor.matmul(po[:], hid[:, k, bass.ts(nt, P)],
                                     w2_sb[:, k, :],
                                     start=(k == 0), stop=(k == HT - 1))
                nc.scalar.activation(o_sb[:, nt, :], po[:], relu)
                if last_e:
                    # tail: store per n-tile so the final store isn't one
                    # serialized 1MB DMA after the last relu (no loads are in
                    # flight now, so sem-lane reuse can't stall anything)
                    nc.scalar.dma_start(out_d[e, bass.ds(nt * P, P), :],
                                        o_sb[:, nt, :])
            if not last_e:
                nc.scalar.dma_start(
                    out_d[e].rearrange("(nt p) o -> p nt o", p=P), o_sb[:])

    nc.compile()
    _CACHE["nc"] = nc
    _CACHE["names"] = ("xt", "w1t", "w2t", "out")
    return nc, _CACHE["names"]


def _prep_inputs(x: np.ndarray, w1: np.ndarray, w2: np.ndarray):
    """Shard across cores + cast bf16 + pre-tile so all DMAs are contiguous."""
    # xT, partition-major: xt[e, p, d, n] = x[e, n, d*128+p]
    xt = np.ascontiguousarray(
        x.astype(_BF16).transpose(0, 2, 1)      # [E, D_IN, N]
        .reshape(E, DT, P, N).transpose(0, 2, 1, 3))  # [E, P, DT, N]
    # w1 h-tiled, partition-major: w1t[e, h, p, dt*128+c] = w1[e, dt*128+p, h*128+c]
    w1t = np.ascontiguousarray(
        w1.astype(_BF16).reshape(E, DT, P, HT, P)
        .transpose(0, 3, 2, 1, 4).reshape(E, HT, P, DT * P))
    # w2 k-tiled, partition-major: w2t[e, p, k, o] = w2[e, k*128+p, o]
    w2t = np.ascontiguousarray(
        w2.astype(_BF16).reshape(E, HT, P, D_OUT).transpose(0, 2, 1, 3))

    in_maps = []
    for c in range(NCORES):
        sl = slice(c * E_PER, (c + 1) * E_PER)
        in_maps.append({"xt": xt[sl], "w1t": w1t[sl], "w2t": w2t[sl]})
    return in_maps


def run(x, w1, w2, trace=False, **trace_kwargs):
    """Run on 8 cores; returns (full_out, BassKernelResults)."""
    from concourse.bass_utils import run_bass_kernel_spmd

    nc, _ = _build_program()
    in_maps = _prep_inputs(np.asarray(x), np.asarray(w1), np.asarray(w2))
    res = run_bass_kernel_spmd(nc, in_maps, list(range(NCORES)), trace=trace,
                               **trace_kwargs)
    out = np.concatenate([res.results[c]["out"] for c in range(NCORES)], axis=0)
    return out, res


def kernel(x: np.ndarray, w1: np.ndarray, w2: np.ndarray) -> np.ndarray:
    out, _ = run(x, w1, w2, trace=False)
    return out
